# revision 1
# baseline (speedup 1.0000x reference)
"""Trainium2 Bass kernel for nn_MixtureOfExperts (B=8192, D=1024, E=12, H=512, O=256).

Strategy:
- Data-parallel over 8 NeuronCores: each core processes 1024 rows of x; all
  weights replicated. Host gathers/concats core outputs.
- Host-side prep: eval-mode BatchNorm (which follows each ReLU) is folded into
  the NEXT layer's weights and bias:  bn(relu(z)) = s*relu(z) + t  with
  s = g/sqrt(v+eps) > 0, t = b - m*s, so
      bn(relu(z)) @ W + c  ==  relu(z) @ (diag(s) W) + (c + t @ W).
  x is pre-transposed and all weights pre-tiled on host into the exact SBUF
  layout ([128 part, chunk, free] with per-partition-contiguous DRAM bytes) so
  every big DMA is a fully contiguous copy (~350 GB/s vs ~139 GB/s scattered).
- All matmuls use fp32r (full PE rate when moving free dim >= 256).
- Layers 1-3 feature-major; layer 4 batch-major (stationary = h3T slice); gate
  prob applied as per-partition scalar on ScalarE, experts accumulated on
  VectorE into acc, which is pre-initialized with sum_e gate_e * bias4_e
  computed via a PE-transposed-gates matmul against the bias matrix.
"""

import numpy as np
from contextlib import ExitStack

import concourse.bass as bass
import concourse.mybir as mybir
import concourse.tile as tile
from concourse import bacc
from concourse.bass import ts
from concourse.bass_utils import run_bass_kernel_spmd

B, D, E, H, O = 8192, 1024, 12, 512, 256
NCORES = 8
BL = B // NCORES          # 1024 batch rows per core
EPS = 1e-5
F32 = mybir.dt.float32
F32R = mybir.dt.float32r
AF = mybir.ActivationFunctionType
ALU = mybir.AluOpType
AX = mybir.AxisListType

DCH = D // 128            # 8  d-chunks
H1CH = H // 128           # 4  h1-chunks
H3CH = (H // 2) // 128    # 2  h3-chunks
BCH = BL // 128           # 8  b-chunks of 128
BH = BL // 512            # 2  b-halves of 512
NB = 512                  # moving free dim for layers 1-3


def _build_bass():
    nc = bacc.Bacc("TRN2", target_bir_lowering=False, debug=False,
                   enable_asserts=False, num_devices=NCORES)

    # DRAM tensors. Weight tensors are host-pre-tiled to [.., 128, ch, free]
    # so the per-expert slab is contiguous and DMAs coalesce.
    xt_d = nc.dram_tensor("xt", [DCH, 128, BL + 256], F32R, kind="ExternalInput")
    w1_d = nc.dram_tensor("w1", [E, 128, DCH, H], F32R, kind="ExternalInput")
    w2_d = nc.dram_tensor("w2", [E, 128, H1CH, H], F32R, kind="ExternalInput")
    w3_d = nc.dram_tensor("w3", [E, 128, H1CH, H // 2], F32R, kind="ExternalInput")
    w4_d = nc.dram_tensor("w4", [E, 128, H3CH, O], F32R, kind="ExternalInput")
    b4_d = nc.dram_tensor("b4", [E, O], F32R, kind="ExternalInput")
    eb_d = nc.dram_tensor("eb", [128, E, 10], F32, kind="ExternalInput")
    # packed small constants:
    #   pkr (f32r) cols: [0:128 ones | 128:384 gw2 | 384:396 gw3 | 396:408 gb3 | 408:664 bmat]
    #   pkf (f32)  cols: [0:2 gb1 | 2:3 gb2 | 3:131 ident]
    pkr_d = nc.dram_tensor("pkr", [128, 664], F32R, kind="ExternalInput")
    pkf_d = nc.dram_tensor("pkf", [128, 131], F32, kind="ExternalInput")
    out_d = nc.dram_tensor("out", [BL, O], F32, kind="ExternalOutput")

    with tile.TileContext(nc) as tc, ExitStack() as ctx:
        const = ctx.enter_context(tc.tile_pool(name="const", bufs=1))
        gatep = ctx.enter_context(tc.tile_pool(name="gatep", bufs=1))
        gtmp = ctx.enter_context(tc.tile_pool(name="gtmp", bufs=2))
        wpool = ctx.enter_context(tc.tile_pool(name="wpool", bufs=2))
        actp = ctx.enter_context(tc.tile_pool(name="actp", bufs=1))
        accp = ctx.enter_context(tc.tile_pool(name="accp", bufs=1))
        tmpp = ctx.enter_context(tc.tile_pool(name="tmpp", bufs=4))
        psA = ctx.enter_context(tc.tile_pool(name="psA", bufs=5, space="PSUM"))
        ps4 = ctx.enter_context(tc.tile_pool(name="ps4", bufs=3, space="PSUM"))

        # ---- PE warmup: keep HAM busy while startup DMAs land ----
        scr = const.tile([128, 256], F32)
        nc.vector.memset(scr, 0.0)
        for r in range(12):
            wps = ps4.tile([128, O], F32, tag="p4")
            nc.tensor.matmul(wps, scr[:, :128], scr, start=True, stop=True)

        # ---- constants / full-lifetime tiles ----
        xtg = const.tile([128, DCH, BL + 256], F32R)
        for dc in range(DCH):
            eng = nc.sync if dc % 2 == 0 else nc.gpsimd
            eng.dma_start(out=xtg[:, dc], in_=xt_d.ap()[dc])
        pkr = const.tile([128, 664], F32R)
        nc.sync.dma_start(out=pkr, in_=pkr_d.ap())
        pkf = const.tile([128, 131], F32)
        nc.sync.dma_start(out=pkf, in_=pkf_d.ap())
        ebt = const.tile([128, E, 10], F32)
        nc.sync.dma_start(out=ebt, in_=eb_d.ap())
        gw2 = pkr[:, 128:384].rearrange("p (c m) -> p c m", c=2)
        gw3 = pkr[:, 384:396]
        gb3 = pkr[:1, 396:408]
        ones = pkr[:1, 0:128]
        bmat = pkr[:E, 408:664]
        gb1 = pkf[:, 0:2]
        gb2 = pkf[:, 2:3]
        ident = pkf[:, 3:131]
        acc = accp.tile([128, BCH, O], F32)

        # ---- gate network ----
        g1t = gatep.tile([128, 2, BL], F32R)
        g2t = gatep.tile([128, BL], F32R)
        gates = gatep.tile([128, BCH, E], F32)
        for bh in range(BH):
            for hc in range(2):
                ps = psA.tile([128, NB], F32)
                for dc in range(DCH):
                    nc.tensor.matmul(ps, xtg[:, dc, BL + hc * 128:BL + hc * 128 + 128],
                                     xtg[:, dc, ts(bh, NB)],
                                     start=(dc == 0), stop=(dc == DCH - 1))
                nc.scalar.activation(g1t[:, hc, ts(bh, NB)], ps, AF.Relu,
                                     bias=gb1[:, hc:hc + 1])
            ps = psA.tile([128, NB], F32)
            for kc in range(2):
                nc.tensor.matmul(ps, gw2[:, kc, :], g1t[:, kc, ts(bh, NB)],
                                 start=(kc == 0), stop=(kc == 1))
            nc.scalar.activation(g2t[:, ts(bh, NB)], ps, AF.Relu, bias=gb2[:, 0:1])
        psgall = ps4.tile([128, BCH, E], F32, tag="p4", name="psgall")
        for bc in range(BCH):
            nc.tensor.matmul(psgall[:, bc, :], g2t[:, ts(bc, 128)], gw3,
                             start=True, stop=False)
            nc.tensor.matmul(psgall[:, bc, :], ones[:1, :], gb3[:1, :],
                             start=False, stop=True)
        exall = gatep.tile([128, BCH, E], F32)
        nc.scalar.activation(exall, psgall, AF.Exp)
        sms = gtmp.tile([128, BCH], F32)
        nc.vector.tensor_reduce(sms, exall, AX.X, ALU.add)
        rcs = gtmp.tile([128, BCH], F32)
        nc.vector.reciprocal(rcs, sms)
        for bc in range(BCH):
            nc.scalar.activation(gates[:, bc, :], exall[:, bc, :], AF.Copy,
                                 scale=rcs[:, bc:bc + 1])

        # ---- init acc with the gate-weighted layer-4 bias: acc = gates @ B ----
        gTall = gatep.tile([E, BCH, 128], F32R)
        for bc in range(BCH):
            gps = ps4.tile([E, 128], F32, tag="p4", name="gps")
            nc.tensor.transpose(gps, gates[:, bc, :], ident)
            nc.scalar.activation(gTall[:, bc, :], gps, AF.Copy)
        for bc in range(BCH):
            bps = ps4.tile([128, O], F32, tag="p4")
            nc.tensor.matmul(bps, gTall[:, bc, :], bmat, start=True, stop=True)
            nc.vector.tensor_copy(acc[:, bc, :], bps)

        # ---- experts ----
        for e in range(E):
            w1t = wpool.tile([128, DCH, H], F32R)
            nc.sync.dma_start(out=w1t[:, :DCH // 2], in_=w1_d.ap()[e, :, :DCH // 2])
            nc.sync.dma_start(out=w1t[:, DCH // 2:], in_=w1_d.ap()[e, :, DCH // 2:])
            w2t = wpool.tile([128, H1CH, H], F32R)
            nc.sync.dma_start(out=w2t, in_=w2_d.ap()[e])
            w3t = wpool.tile([128, H1CH, H // 2], F32R)
            nc.sync.dma_start(out=w3t, in_=w3_d.ap()[e])
            w4t = wpool.tile([128, H3CH, O], F32R)
            nc.sync.dma_start(out=w4t, in_=w4_d.ap()[e])

            h1t = actp.tile([128, H1CH, BL], F32R)
            h2t = actp.tile([128, H1CH, BL], F32R)
            h3t = actp.tile([128, H3CH, BL], F32R)

            for bh in range(BH):            # layer 1: [1024] -> [512]
                for hc in range(H1CH):
                    ps = psA.tile([128, NB], F32)
                    for dc in range(DCH):
                        nc.tensor.matmul(ps, w1t[:, dc, ts(hc, 128)],
                                         xtg[:, dc, ts(bh, NB)],
                                         start=(dc == 0), stop=(dc == DCH - 1))
                    nc.vector.tensor_scalar(h1t[:, hc, ts(bh, NB)], ps,
                                            ebt[:, e, hc:hc + 1], 0.0,
                                            ALU.add, ALU.max)
            for bh in range(BH):            # layer 2: [512] -> [512]
                for hc in range(H1CH):
                    ps = psA.tile([128, NB], F32)
                    for kc in range(H1CH):
                        nc.tensor.matmul(ps, w2t[:, kc, ts(hc, 128)], h1t[:, kc, ts(bh, NB)],
                                         start=(kc == 0), stop=(kc == H1CH - 1))
                    nc.scalar.activation(h2t[:, hc, ts(bh, NB)], ps, AF.Relu,
                                         bias=ebt[:, e, 4 + hc:5 + hc])
            for bh in range(BH):            # layer 3: [512] -> [256]
                for hc in range(H3CH):
                    ps = psA.tile([128, NB], F32)
                    for kc in range(H1CH):
                        nc.tensor.matmul(ps, w3t[:, kc, ts(hc, 128)], h2t[:, kc, ts(bh, NB)],
                                         start=(kc == 0), stop=(kc == H1CH - 1))
                    nc.scalar.activation(h3t[:, hc, ts(bh, NB)], ps, AF.Relu,
                                         bias=ebt[:, e, 8 + hc:9 + hc])
            for bc in range(BCH):           # layer 4 + gated accumulation
                p4 = ps4.tile([128, O], F32, tag="p4")
                nc.tensor.matmul(p4, h3t[:, 0, ts(bc, 128)], w4t[:, 0, :],
                                 start=True, stop=False)
                nc.tensor.matmul(p4, h3t[:, 1, ts(bc, 128)], w4t[:, 1, :],
                                 start=False, stop=True)
                tm = tmpp.tile([128, O], F32)
                nc.scalar.activation(tm, p4, AF.Copy, scale=gates[:, bc, e:e + 1])
                nc.vector.tensor_add(acc[:, bc, :], acc[:, bc, :], tm)

        for bc in range(BCH):
            nc.sync.dma_start(out=out_d.ap()[ts(bc, 128), :], in_=acc[:, bc, :])

    nc.compile()
    return nc


def _tile128(w):
    """[K, N] -> [128, K//128, N] with per-partition-contiguous bytes."""
    k, n = w.shape
    return np.ascontiguousarray(w.reshape(k // 128, 128, n).transpose(1, 0, 2))


def _fold(inputs):
    """Fold BatchNorms into next-layer weights/biases (float64 for exactness)."""
    f = {k: np.asarray(v, dtype=np.float64) for k, v in inputs.items()}

    def sb(g, b, m, v):
        s = g / np.sqrt(v + EPS)
        return s, b - m * s

    out = {}
    # gate
    sg1, tg1 = sb(f["gbn1_g"], f["gbn1_b"], f["gbn1_m"], f["gbn1_v"])
    sg2, tg2 = sb(f["gbn2_g"], f["gbn2_b"], f["gbn2_m"], f["gbn2_v"])
    gw1t = _tile128(f["gw1"])                     # [128, DCH, 256]
    gb1c = f["gb1"]
    gw2t = _tile128(sg1[:, None] * f["gw2"])      # [128, 2, 128]
    gb2c = f["gb2"] + tg1 @ f["gw2"]
    gw3t = sg2[:, None] * f["gw3"]                # [128, E]
    gb3r = f["gb3"] + tg2 @ f["gw3"]
    out["_gw1t"] = gw1t
    # experts
    s1, t1 = sb(f["ebn1_g"], f["ebn1_b"], f["ebn1_m"], f["ebn1_v"])   # [E,H]
    s2, t2 = sb(f["ebn2_g"], f["ebn2_b"], f["ebn2_m"], f["ebn2_v"])   # [E,H]
    s3, t3 = sb(f["ebn3_g"], f["ebn3_b"], f["ebn3_m"], f["ebn3_v"])   # [E,H/2]
    out["w1"] = np.stack([_tile128(f["ew1"][e]) for e in range(E)])
    b1 = f["eb1"]                                                     # [E,H]
    out["w2"] = np.stack([_tile128(s1[e][:, None] * f["ew2"][e]) for e in range(E)])
    b2 = f["eb2"] + np.einsum("eh,eho->eo", t1, f["ew2"])
    out["w3"] = np.stack([_tile128(s2[e][:, None] * f["ew3"][e]) for e in range(E)])
    b3 = f["eb3"] + np.einsum("eh,eho->eo", t2, f["ew3"])
    out["w4"] = np.stack([_tile128(s3[e][:, None] * f["ew4"][e]) for e in range(E)])
    out["b4"] = f["eb4"] + np.einsum("eh,eho->eo", t3, f["ew4"])
    # packed activation-bias columns: [E, 128, 10]
    eb = np.zeros((E, 128, 10))
    eb[:, :, 0:4] = b1.reshape(E, 4, 128).transpose(0, 2, 1)
    eb[:, :, 4:8] = b2.reshape(E, 4, 128).transpose(0, 2, 1)
    eb[:, :, 8:10] = b3.reshape(E, 2, 128).transpose(0, 2, 1)
    out["eb"] = eb.transpose(1, 0, 2)             # [128, E, 10]
    pkr = np.zeros((128, 664))
    pkr[:1, 0:128] = 1.0                          # ones row
    pkr[:, 128:384] = gw2t.reshape(128, 256)
    pkr[:, 384:396] = gw3t
    pkr[:1, 396:408] = gb3r
    pkr[:E, 408:664] = out["b4"]
    out["pkr"] = pkr
    pkf = np.zeros((128, 131))
    pkf[:, 0:2] = gb1c.reshape(2, 128).T
    pkf[:, 2:3] = gb2c.reshape(1, 128).T
    pkf[:, 3:131] = np.eye(128)
    out["pkf"] = pkf
    return {k: np.ascontiguousarray(v, dtype=np.float32) for k, v in out.items()}


_CACHE = {}


def build_in_maps(inputs):
    w = _fold(inputs)
    gw1t = w.pop("_gw1t").transpose(1, 0, 2)                            # [DCH, 128, 256]
    xt_full = np.asarray(inputs["x"], dtype=np.float32).T               # [D, B]
    in_maps = []
    for c in range(NCORES):
        m = dict(w)
        xtg = np.empty((DCH, 128, BL + 256), dtype=np.float32)
        xtg[:, :, :BL] = xt_full[:, c * BL:(c + 1) * BL].reshape(DCH, 128, BL)
        xtg[:, :, BL:] = gw1t
        m["xt"] = xtg
        in_maps.append(m)

    return in_maps


def kernel(**inputs) -> np.ndarray:
    if "nc" not in _CACHE:
        _CACHE["nc"] = _build_bass()
    nc = _CACHE["nc"]

    in_maps = build_in_maps(inputs)
    res = run_bass_kernel_spmd(nc, in_maps, core_ids=list(range(NCORES)))
    return np.concatenate([r["out"] for r in res.results], axis=0)



# revision 2
# speedup vs baseline: 1.0955x; 1.0955x over previous
"""Trainium2 Bass kernel for nn_MixtureOfExperts (B=8192, D=1024, E=12, H=512, O=256).

Strategy:
- Data-parallel over 8 NeuronCores: each core processes 1024 rows of x; all
  weights replicated. Host gathers/concats core outputs.
- Host-side prep: eval-mode BatchNorm (which follows each ReLU) is folded into
  the NEXT layer's weights and bias:  bn(relu(z)) = s*relu(z) + t  with
  s = g/sqrt(v+eps) > 0, t = b - m*s, so
      bn(relu(z)) @ W + c  ==  relu(z) @ (diag(s) W) + (c + t @ W).
  x is pre-transposed and all weights pre-tiled on host into the exact SBUF
  layout ([128 part, chunk, free] with per-partition-contiguous DRAM bytes) so
  every big DMA is a fully contiguous copy.
- All matmul operands in bf16 (same PE stream rate as fp32r, half the DMA
  bytes and LDWEIGHTS time); PSUM accumulation and bias/softmax math in fp32.
- Layers 1-3 feature-major; layer 4 batch-major (stationary = h3T slice); gate
  prob applied as per-partition scalar on ScalarE, experts accumulated on
  VectorE into acc, which is pre-initialized with sum_e gate_e * bias4_e
  computed via a PE-transposed-gates matmul against the bias matrix.
- Softmax-dependent PE work (gate transposes + acc init) is deferred until
  after expert-0 layer 1 so the PE never stalls on the softmax chain; the
  last expert's layer-4 results are DMA'd out per batch-chunk to hide the
  output-store tail behind compute.
"""

import numpy as np
import ml_dtypes
from contextlib import ExitStack

import concourse.bass as bass
import concourse.mybir as mybir
import concourse.tile as tile
from concourse import bacc
from concourse.bass import ts
from concourse.bass_utils import run_bass_kernel_spmd

B, D, E, H, O = 8192, 1024, 12, 512, 256
NCORES = 8
BL = B // NCORES          # 1024 batch rows per core
EPS = 1e-5
F32 = mybir.dt.float32
BF16 = mybir.dt.bfloat16
AF = mybir.ActivationFunctionType
ALU = mybir.AluOpType
AX = mybir.AxisListType

DCH = D // 128            # 8  d-chunks
H1CH = H // 128           # 4  h1-chunks
H3CH = (H // 2) // 128    # 2  h3-chunks
BCH = BL // 128           # 8  b-chunks of 128
BH = BL // 512            # 2  b-halves of 512
NB = 512                  # moving free dim for layers 1-3


def _build_bass():
    nc = bacc.Bacc("TRN2", target_bir_lowering=False, debug=False,
                   enable_asserts=False, num_devices=NCORES)

    # DRAM tensors. Weight tensors are host-pre-tiled to [.., 128, ch, free]
    # so the per-expert slab is contiguous and DMAs coalesce. All matmul
    # operand tensors are bf16.
    xt_d = nc.dram_tensor("xt", [DCH, 128, BL + 256], BF16, kind="ExternalInput")
    w1_d = nc.dram_tensor("w1", [E, 128, DCH, H], BF16, kind="ExternalInput")
    w2_d = nc.dram_tensor("w2", [E, 128, H1CH, H], BF16, kind="ExternalInput")
    w3_d = nc.dram_tensor("w3", [E, 128, H1CH, H // 2], BF16, kind="ExternalInput")
    w4_d = nc.dram_tensor("w4", [E, 128, H3CH, O], BF16, kind="ExternalInput")
    eb_d = nc.dram_tensor("eb", [128, E, 10], F32, kind="ExternalInput")
    # packed small constants:
    #   pkr (bf16) cols: [0:128 ones | 128:384 gw2 | 384:396 gw3 | 396:408 gb3 | 408:664 bmat]
    #   pkf (f32)  cols: [0:2 gb1 | 2:3 gb2 | 3:131 ident]
    pkr_d = nc.dram_tensor("pkr", [128, 664], BF16, kind="ExternalInput")
    pkf_d = nc.dram_tensor("pkf", [128, 131], F32, kind="ExternalInput")
    out_d = nc.dram_tensor("out", [BL, O], F32, kind="ExternalOutput")

    with tile.TileContext(nc) as tc, ExitStack() as ctx:
        const = ctx.enter_context(tc.tile_pool(name="const", bufs=1))
        gatep = ctx.enter_context(tc.tile_pool(name="gatep", bufs=1))
        gtmp = ctx.enter_context(tc.tile_pool(name="gtmp", bufs=2))
        wpool = ctx.enter_context(tc.tile_pool(name="wpool", bufs=3))
        actp = ctx.enter_context(tc.tile_pool(name="actp", bufs=1))
        accp = ctx.enter_context(tc.tile_pool(name="accp", bufs=1))
        tmpp = ctx.enter_context(tc.tile_pool(name="tmpp", bufs=4))
        psA = ctx.enter_context(tc.tile_pool(name="psA", bufs=4, space="PSUM"))
        ps4 = ctx.enter_context(tc.tile_pool(name="ps4", bufs=4, space="PSUM"))

        # ---- constants / full-lifetime tiles; queue split keeps both DMA
        # rings loaded evenly and lands expert-0/1 weights early ----
        pkr = const.tile([128, 664], BF16)
        nc.sync.dma_start(out=pkr, in_=pkr_d.ap())
        pkf = const.tile([128, 131], F32)
        nc.gpsimd.dma_start(out=pkf, in_=pkf_d.ap())
        ebt = const.tile([128, E, 10], F32)
        nc.gpsimd.dma_start(out=ebt, in_=eb_d.ap())
        xtg = const.tile([128, DCH, BL + 256], BF16)
        for dc in range(DCH):
            eng = nc.sync if dc % 2 == 0 else nc.gpsimd
            eng.dma_start(out=xtg[:, dc], in_=xt_d.ap()[dc])

        gw2 = pkr[:, 128:384].rearrange("p (c m) -> p c m", c=2)
        gw3 = pkr[:, 384:396]
        gb3 = pkr[:1, 396:408]
        ones = pkr[:1, 0:128]
        bmat = pkr[:E, 408:664]
        gb1 = pkf[:, 0:2]
        gb2 = pkf[:, 2:3]
        ident = pkf[:, 3:131]
        acc = accp.tile([128, BCH, O], F32)

        # ---- PE warmup: keep HAM busy while startup DMAs land ----
        scr = const.tile([128, 256], F32)
        nc.vector.memset(scr, 0.0)
        for r in range(6):
            wps = ps4.tile([128, O], F32, tag="p4")
            nc.tensor.matmul(wps, scr[:, :128], scr, start=True, stop=True)

        # ---- gate network (layers 1-3; softmax tail deferred) ----
        g1t = gatep.tile([128, 2, BL], BF16)
        g2t = gatep.tile([128, BL], BF16)
        gates = gatep.tile([128, BCH, E], F32)
        for bh in range(BH):
            for hc in range(2):
                ps = psA.tile([128, NB], F32)
                for dc in range(DCH):
                    nc.tensor.matmul(ps, xtg[:, dc, BL + hc * 128:BL + hc * 128 + 128],
                                     xtg[:, dc, ts(bh, NB)],
                                     start=(dc == 0), stop=(dc == DCH - 1))
                nc.scalar.activation(g1t[:, hc, ts(bh, NB)], ps, AF.Relu,
                                     bias=gb1[:, hc:hc + 1])
            ps = psA.tile([128, NB], F32)
            for kc in range(2):
                nc.tensor.matmul(ps, gw2[:, kc, :], g1t[:, kc, ts(bh, NB)],
                                 start=(kc == 0), stop=(kc == 1))
            nc.scalar.activation(g2t[:, ts(bh, NB)], ps, AF.Relu, bias=gb2[:, 0:1])
        psgall = ps4.tile([128, BCH, E], F32, tag="p4", name="psgall")
        for bc in range(BCH):
            nc.tensor.matmul(psgall[:, bc, :], g2t[:, ts(bc, 128)], gw3,
                             start=True, stop=False)
            nc.tensor.matmul(psgall[:, bc, :], ones[:1, :], gb3[:1, :],
                             start=False, stop=True)
        exall = gatep.tile([128, BCH, E], F32)
        nc.scalar.activation(exall, psgall, AF.Exp)
        sms = gtmp.tile([128, BCH], F32)
        nc.vector.tensor_reduce(sms, exall, AX.X, ALU.add)
        rcs = gtmp.tile([128, BCH], F32)
        nc.vector.reciprocal(rcs, sms)
        for bc in range(BCH):
            nc.scalar.activation(gates[:, bc, :], exall[:, bc, :], AF.Copy,
                                 scale=rcs[:, bc:bc + 1])
        gTall = gatep.tile([E, BCH, 128], BF16)

        # ---- experts ----
        for e in range(E):
            w1t = wpool.tile([128, DCH, H], BF16)
            nc.sync.dma_start(out=w1t[:, :DCH // 2], in_=w1_d.ap()[e, :, :DCH // 2])
            nc.gpsimd.dma_start(out=w1t[:, DCH // 2:], in_=w1_d.ap()[e, :, DCH // 2:])
            w2t = wpool.tile([128, H1CH, H], BF16)
            nc.sync.dma_start(out=w2t, in_=w2_d.ap()[e])
            w3t = wpool.tile([128, H1CH, H // 2], BF16)
            nc.gpsimd.dma_start(out=w3t, in_=w3_d.ap()[e])
            w4t = wpool.tile([128, H3CH, O], BF16)
            nc.gpsimd.dma_start(out=w4t, in_=w4_d.ap()[e])

            h1t = actp.tile([128, H1CH, BL], BF16)
            h2t = actp.tile([128, H1CH, BL], BF16)
            h3t = actp.tile([128, H3CH, BL], BF16)

            for bh in range(BH):            # layer 1: [1024] -> [512]
                for hc in range(H1CH):
                    ps = psA.tile([128, NB], F32)
                    for dc in range(DCH):
                        nc.tensor.matmul(ps, w1t[:, dc, ts(hc, 128)],
                                         xtg[:, dc, ts(bh, NB)],
                                         start=(dc == 0), stop=(dc == DCH - 1))
                    nc.vector.tensor_scalar(h1t[:, hc, ts(bh, NB)], ps,
                                            ebt[:, e, hc:hc + 1], 0.0,
                                            ALU.add, ALU.max)

            if e == 0:
                # softmax-dependent PE tail, scheduled here so the PE queue
                # never waits on the scalar/vector softmax chain: transpose
                # gates and init acc with the gate-weighted layer-4 bias
                # (acc = gates @ B).
                for bc in range(BCH):
                    gps = ps4.tile([E, 128], F32, tag="p4", name="gps")
                    nc.tensor.transpose(gps, gates[:, bc, :], ident)
                    nc.scalar.activation(gTall[:, bc, :], gps, AF.Copy)
                for bc in range(BCH):
                    bps = ps4.tile([128, O], F32, tag="p4")
                    nc.tensor.matmul(bps, gTall[:, bc, :], bmat, start=True, stop=True)
                    nc.vector.tensor_copy(acc[:, bc, :], bps)

            for bh in range(BH):            # layer 2: [512] -> [512]
                for hc in range(H1CH):
                    ps = psA.tile([128, NB], F32)
                    for kc in range(H1CH):
                        nc.tensor.matmul(ps, w2t[:, kc, ts(hc, 128)], h1t[:, kc, ts(bh, NB)],
                                         start=(kc == 0), stop=(kc == H1CH - 1))
                    nc.scalar.activation(h2t[:, hc, ts(bh, NB)], ps, AF.Relu,
                                         bias=ebt[:, e, 4 + hc:5 + hc])
            for bh in range(BH):            # layer 3: [512] -> [256]
                for hc in range(H3CH):
                    ps = psA.tile([128, NB], F32)
                    for kc in range(H1CH):
                        nc.tensor.matmul(ps, w3t[:, kc, ts(hc, 128)], h2t[:, kc, ts(bh, NB)],
                                         start=(kc == 0), stop=(kc == H1CH - 1))
                    nc.scalar.activation(h3t[:, hc, ts(bh, NB)], ps, AF.Relu,
                                         bias=ebt[:, e, 8 + hc:9 + hc])
            for bc in range(BCH):           # layer 4 + gated accumulation
                p4 = ps4.tile([128, O], F32, tag="p4")
                nc.tensor.matmul(p4, h3t[:, 0, ts(bc, 128)], w4t[:, 0, :],
                                 start=True, stop=False)
                nc.tensor.matmul(p4, h3t[:, 1, ts(bc, 128)], w4t[:, 1, :],
                                 start=False, stop=True)
                tm = tmpp.tile([128, O], F32)
                nc.scalar.activation(tm, p4, AF.Copy, scale=gates[:, bc, e:e + 1])
                nc.vector.tensor_add(acc[:, bc, :], acc[:, bc, :], tm)
                if e == E - 1:
                    # stream results out as each chunk finalizes to hide the
                    # store tail behind the remaining layer-4 chunks
                    nc.sync.dma_start(out=out_d.ap()[ts(bc, 128), :], in_=acc[:, bc, :])

    nc.compile()
    return nc


def _tile128(w):
    """[K, N] -> [128, K//128, N] with per-partition-contiguous bytes."""
    k, n = w.shape
    return np.ascontiguousarray(w.reshape(k // 128, 128, n).transpose(1, 0, 2))


def _fold(inputs):
    """Fold BatchNorms into next-layer weights/biases (float64 for exactness)."""
    f = {k: np.asarray(v, dtype=np.float64) for k, v in inputs.items()}

    def sb(g, b, m, v):
        s = g / np.sqrt(v + EPS)
        return s, b - m * s

    out = {}
    # gate
    sg1, tg1 = sb(f["gbn1_g"], f["gbn1_b"], f["gbn1_m"], f["gbn1_v"])
    sg2, tg2 = sb(f["gbn2_g"], f["gbn2_b"], f["gbn2_m"], f["gbn2_v"])
    gw1t = _tile128(f["gw1"])                     # [128, DCH, 256]
    gb1c = f["gb1"]
    gw2t = _tile128(sg1[:, None] * f["gw2"])      # [128, 2, 128]
    gb2c = f["gb2"] + tg1 @ f["gw2"]
    gw3t = sg2[:, None] * f["gw3"]                # [128, E]
    gb3r = f["gb3"] + tg2 @ f["gw3"]
    out["_gw1t"] = gw1t
    # experts
    s1, t1 = sb(f["ebn1_g"], f["ebn1_b"], f["ebn1_m"], f["ebn1_v"])   # [E,H]
    s2, t2 = sb(f["ebn2_g"], f["ebn2_b"], f["ebn2_m"], f["ebn2_v"])   # [E,H]
    s3, t3 = sb(f["ebn3_g"], f["ebn3_b"], f["ebn3_m"], f["ebn3_v"])   # [E,H/2]
    out["w1"] = np.stack([_tile128(f["ew1"][e]) for e in range(E)])
    b1 = f["eb1"]                                                     # [E,H]
    out["w2"] = np.stack([_tile128(s1[e][:, None] * f["ew2"][e]) for e in range(E)])
    b2 = f["eb2"] + np.einsum("eh,eho->eo", t1, f["ew2"])
    out["w3"] = np.stack([_tile128(s2[e][:, None] * f["ew3"][e]) for e in range(E)])
    b3 = f["eb3"] + np.einsum("eh,eho->eo", t2, f["ew3"])
    out["w4"] = np.stack([_tile128(s3[e][:, None] * f["ew4"][e]) for e in range(E)])
    b4 = f["eb4"] + np.einsum("eh,eho->eo", t3, f["ew4"])
    # packed activation-bias columns: [E, 128, 10]
    eb = np.zeros((E, 128, 10))
    eb[:, :, 0:4] = b1.reshape(E, 4, 128).transpose(0, 2, 1)
    eb[:, :, 4:8] = b2.reshape(E, 4, 128).transpose(0, 2, 1)
    eb[:, :, 8:10] = b3.reshape(E, 2, 128).transpose(0, 2, 1)
    out["eb"] = eb.transpose(1, 0, 2)             # [128, E, 10]
    pkr = np.zeros((128, 664))
    pkr[:1, 0:128] = 1.0                          # ones row
    pkr[:, 128:384] = gw2t.reshape(128, 256)
    pkr[:, 384:396] = gw3t
    pkr[:1, 396:408] = gb3r
    pkr[:E, 408:664] = b4
    out["pkr"] = pkr
    pkf = np.zeros((128, 131))
    pkf[:, 0:2] = gb1c.reshape(2, 128).T
    pkf[:, 2:3] = gb2c.reshape(1, 128).T
    pkf[:, 3:131] = np.eye(128)
    out["pkf"] = pkf

    res = {}
    for k, v in out.items():
        dt = np.float32 if k in ("eb", "pkf") else ml_dtypes.bfloat16
        res[k] = np.ascontiguousarray(v, dtype=dt) if k != "_gw1t" else v
    return res


_CACHE = {}


def build_in_maps(inputs):
    w = _fold(inputs)
    gw1t = w.pop("_gw1t").transpose(1, 0, 2)                            # [DCH, 128, 256]
    xt_full = np.asarray(inputs["x"], dtype=np.float32).T               # [D, B]
    in_maps = []
    for c in range(NCORES):
        m = dict(w)
        xtg = np.empty((DCH, 128, BL + 256), dtype=ml_dtypes.bfloat16)
        xtg[:, :, :BL] = xt_full[:, c * BL:(c + 1) * BL].reshape(DCH, 128, BL).astype(ml_dtypes.bfloat16)
        xtg[:, :, BL:] = gw1t.astype(ml_dtypes.bfloat16)
        m["xt"] = xtg
        in_maps.append(m)

    return in_maps


def kernel(**inputs) -> np.ndarray:
    if "nc" not in _CACHE:
        _CACHE["nc"] = _build_bass()
    nc = _CACHE["nc"]

    in_maps = build_in_maps(inputs)
    res = run_bass_kernel_spmd(nc, in_maps, core_ids=list(range(NCORES)))
    return np.concatenate([r["out"] for r in res.results], axis=0)


# revision 5
# speedup vs baseline: 1.0995x; 1.0036x over previous
"""Trainium2 Bass kernel for nn_MixtureOfExperts (B=8192, D=1024, E=12, H=512, O=256).

Strategy:
- Data-parallel over 8 NeuronCores: each core processes 1024 rows of x; all
  weights replicated. Host gathers/concats core outputs.
- Host-side prep: eval-mode BatchNorm (which follows each ReLU) is folded into
  the NEXT layer's weights and bias:  bn(relu(z)) = s*relu(z) + t  with
  s = g/sqrt(v+eps) > 0, t = b - m*s, so
      bn(relu(z)) @ W + c  ==  relu(z) @ (diag(s) W) + (c + t @ W).
  x is pre-transposed and all weights pre-tiled on host into the exact SBUF
  layout ([128 part, chunk, free] with per-partition-contiguous DRAM bytes) so
  every big DMA is a fully contiguous copy.
- All matmul operands in bf16 (same PE stream rate as fp32r, half the DMA
  bytes and LDWEIGHTS time); PSUM accumulation and bias/softmax math in fp32.
- Layers 1-3 feature-major; layer 4 batch-major (stationary = h3T slice); gate
  prob applied as per-partition scalar on ScalarE, experts accumulated on
  VectorE into acc, which is pre-initialized with sum_e gate_e * bias4_e
  computed via a PE-transposed-gates matmul against the bias matrix.
- Softmax-dependent PE work (gate transposes + acc init) is deferred until
  after expert-0 layer 1 so the PE never stalls on the softmax chain; the
  last expert's layer-4 results are DMA'd out per batch-chunk to hide the
  output-store tail behind compute.
"""

import numpy as np
import ml_dtypes
from contextlib import ExitStack

import concourse.bass as bass
import concourse.mybir as mybir
import concourse.tile as tile
from concourse import bacc
from concourse.bass import ts
from concourse.bass_utils import run_bass_kernel_spmd

B, D, E, H, O = 8192, 1024, 12, 512, 256
NCORES = 8
BL = B // NCORES          # 1024 batch rows per core
EPS = 1e-5
F32 = mybir.dt.float32
BF16 = mybir.dt.bfloat16
AF = mybir.ActivationFunctionType
ALU = mybir.AluOpType
AX = mybir.AxisListType

DCH = D // 128            # 8  d-chunks
H1CH = H // 128           # 4  h1-chunks
H3CH = (H // 2) // 128    # 2  h3-chunks
BCH = BL // 128           # 8  b-chunks of 128
BH = BL // 512            # 2  b-halves of 512
NB = 512                  # moving free dim for layers 1-3


def _build_bass():
    nc = bacc.Bacc("TRN2", target_bir_lowering=False, debug=False,
                   enable_asserts=False, num_devices=NCORES)

    # DRAM tensors. Weight tensors are host-pre-tiled to [.., 128, ch, free]
    # so the per-expert slab is contiguous and DMAs coalesce. All matmul
    # operand tensors are bf16.
    xt_d = nc.dram_tensor("xt", [DCH, 128, BL + 256], BF16, kind="ExternalInput")
    w1_d = nc.dram_tensor("w1", [E, 128, DCH, H], BF16, kind="ExternalInput")
    w2_d = nc.dram_tensor("w2", [E, 128, H1CH, H], BF16, kind="ExternalInput")
    w3_d = nc.dram_tensor("w3", [E, 128, H1CH, H // 2], BF16, kind="ExternalInput")
    w4_d = nc.dram_tensor("w4", [E, 128, H3CH, O], BF16, kind="ExternalInput")
    eb_d = nc.dram_tensor("eb", [128, E, 10], F32, kind="ExternalInput")
    # packed small constants:
    #   pkr (bf16) cols: [0:128 ones | 128:384 gw2 | 384:396 gw3 | 396:408 gb3 | 408:664 bmat]
    #   pkf (f32)  cols: [0:2 gb1 | 2:3 gb2 | 3:131 ident]
    pkr_d = nc.dram_tensor("pkr", [128, 664], BF16, kind="ExternalInput")
    pkf_d = nc.dram_tensor("pkf", [128, 131], F32, kind="ExternalInput")
    out_d = nc.dram_tensor("out", [BL, O], F32, kind="ExternalOutput")

    with tile.TileContext(nc) as tc, ExitStack() as ctx:
        const = ctx.enter_context(tc.tile_pool(name="const", bufs=1))
        gatep = ctx.enter_context(tc.tile_pool(name="gatep", bufs=1))
        gtmp = ctx.enter_context(tc.tile_pool(name="gtmp", bufs=2))
        wpool = ctx.enter_context(tc.tile_pool(name="wpool", bufs=3))
        actp = ctx.enter_context(tc.tile_pool(name="actp", bufs=1))
        accp = ctx.enter_context(tc.tile_pool(name="accp", bufs=1))
        psA = ctx.enter_context(tc.tile_pool(name="psA", bufs=4, space="PSUM"))
        ps4 = ctx.enter_context(tc.tile_pool(name="ps4", bufs=4, space="PSUM"))

        # ---- constants / full-lifetime tiles; queue split keeps both DMA
        # rings loaded evenly and lands expert-0/1 weights early ----
        pkr = const.tile([128, 664], BF16)
        nc.sync.dma_start(out=pkr, in_=pkr_d.ap())
        pkf = const.tile([128, 131], F32)
        nc.gpsimd.dma_start(out=pkf, in_=pkf_d.ap())
        ebt = const.tile([128, E, 10], F32)
        nc.gpsimd.dma_start(out=ebt, in_=eb_d.ap())
        xtg = const.tile([128, DCH, BL + 256], BF16)
        for dc in range(DCH):
            eng = nc.sync if dc % 2 == 0 else nc.gpsimd
            eng.dma_start(out=xtg[:, dc], in_=xt_d.ap()[dc])

        gw2 = pkr[:, 128:384].rearrange("p (c m) -> p c m", c=2)
        gw3 = pkr[:, 384:396]
        gb3 = pkr[:1, 396:408]
        ones = pkr[:1, 0:128]
        bmat = pkr[:E, 408:664]
        gb1 = pkf[:, 0:2]
        gb2 = pkf[:, 2:3]
        ident = pkf[:, 3:131]
        acc = accp.tile([128, BCH, O], F32)

        # ---- gate network (layers 1-3; softmax tail deferred) ----
        g1t = gatep.tile([128, 2, BL], BF16)
        g2t = gatep.tile([128, BL], BF16)
        gates = gatep.tile([128, BCH, E], F32)
        for bh in range(BH):
            for hc in range(2):
                ps = psA.tile([128, NB], F32)
                for dc in range(DCH):
                    nc.tensor.matmul(ps, xtg[:, dc, BL + hc * 128:BL + hc * 128 + 128],
                                     xtg[:, dc, ts(bh, NB)],
                                     start=(dc == 0), stop=(dc == DCH - 1))
                nc.scalar.activation(g1t[:, hc, ts(bh, NB)], ps, AF.Relu,
                                     bias=gb1[:, hc:hc + 1])
            ps = psA.tile([128, NB], F32)
            for kc in range(2):
                nc.tensor.matmul(ps, gw2[:, kc, :], g1t[:, kc, ts(bh, NB)],
                                 start=(kc == 0), stop=(kc == 1))
            nc.scalar.activation(g2t[:, ts(bh, NB)], ps, AF.Relu, bias=gb2[:, 0:1])
        psgall = ps4.tile([128, BCH, E], F32, tag="p4", name="psgall")
        for bc in range(BCH):
            nc.tensor.matmul(psgall[:, bc, :], g2t[:, ts(bc, 128)], gw3,
                             start=True, stop=False)
            nc.tensor.matmul(psgall[:, bc, :], ones[:1, :], gb3[:1, :],
                             start=False, stop=True)
        exall = gatep.tile([128, BCH, E], F32)
        nc.scalar.activation(exall, psgall, AF.Exp)
        sms = gtmp.tile([128, BCH], F32)
        nc.vector.tensor_reduce(sms, exall, AX.X, ALU.add)
        rcs = gtmp.tile([128, BCH], F32)
        nc.vector.reciprocal(rcs, sms)
        for bc in range(BCH):
            nc.scalar.activation(gates[:, bc, :], exall[:, bc, :], AF.Copy,
                                 scale=rcs[:, bc:bc + 1])
        gTall = gatep.tile([E, BCH, 128], BF16)

        # ---- experts ----
        for e in range(E):
            w1t = wpool.tile([128, DCH, H], BF16)
            nc.sync.dma_start(out=w1t[:, :DCH // 2], in_=w1_d.ap()[e, :, :DCH // 2])
            nc.gpsimd.dma_start(out=w1t[:, DCH // 2:], in_=w1_d.ap()[e, :, DCH // 2:])
            w2t = wpool.tile([128, H1CH, H], BF16)
            nc.sync.dma_start(out=w2t, in_=w2_d.ap()[e])
            w3t = wpool.tile([128, H1CH, H // 2], BF16)
            nc.gpsimd.dma_start(out=w3t, in_=w3_d.ap()[e])
            w4t = wpool.tile([128, H3CH, O], BF16)
            nc.gpsimd.dma_start(out=w4t, in_=w4_d.ap()[e])

            h1t = actp.tile([128, H1CH, BL], BF16)
            h2t = actp.tile([128, H1CH, BL], BF16)
            h3t = actp.tile([128, H3CH, BL], BF16)

            for bh in range(BH):            # layer 1: [1024] -> [512]
                for hc in range(H1CH):
                    ps = psA.tile([128, NB], F32)
                    for dc in range(DCH):
                        nc.tensor.matmul(ps, w1t[:, dc, ts(hc, 128)],
                                         xtg[:, dc, ts(bh, NB)],
                                         start=(dc == 0), stop=(dc == DCH - 1))
                    nc.vector.tensor_scalar(h1t[:, hc, ts(bh, NB)], ps,
                                            ebt[:, e, hc:hc + 1], 0.0,
                                            ALU.add, ALU.max)

            if e == 0:
                # softmax-dependent PE tail, scheduled here so the PE queue
                # never waits on the scalar/vector softmax chain: transpose
                # gates and init acc with the gate-weighted layer-4 bias
                # (acc = gates @ B).
                for bc in range(BCH):
                    gps = ps4.tile([E, 128], F32, tag="p4", name="gps")
                    nc.tensor.transpose(gps, gates[:, bc, :], ident)
                    nc.scalar.activation(gTall[:, bc, :], gps, AF.Copy)
                for bc in range(BCH):
                    bps = ps4.tile([128, O], F32, tag="p4")
                    nc.tensor.matmul(bps, gTall[:, bc, :], bmat, start=True, stop=True)
                    nc.vector.tensor_copy(acc[:, bc, :], bps)

            for bh in range(BH):            # layer 2: [512] -> [512]
                for hc in range(H1CH):
                    ps = psA.tile([128, NB], F32)
                    for kc in range(H1CH):
                        nc.tensor.matmul(ps, w2t[:, kc, ts(hc, 128)], h1t[:, kc, ts(bh, NB)],
                                         start=(kc == 0), stop=(kc == H1CH - 1))
                    nc.scalar.activation(h2t[:, hc, ts(bh, NB)], ps, AF.Relu,
                                         bias=ebt[:, e, 4 + hc:5 + hc])
            for bh in range(BH):            # layer 3: [512] -> [256]
                for hc in range(H3CH):
                    ps = psA.tile([128, NB], F32)
                    for kc in range(H1CH):
                        nc.tensor.matmul(ps, w3t[:, kc, ts(hc, 128)], h2t[:, kc, ts(bh, NB)],
                                         start=(kc == 0), stop=(kc == H1CH - 1))
                    nc.scalar.activation(h3t[:, hc, ts(bh, NB)], ps, AF.Relu,
                                         bias=ebt[:, e, 8 + hc:9 + hc])
            for bc in range(BCH):           # layer 4 + gated accumulation
                p4 = ps4.tile([128, O], F32, tag="p4")
                nc.tensor.matmul(p4, h3t[:, 0, ts(bc, 128)], w4t[:, 0, :],
                                 start=True, stop=False)
                nc.tensor.matmul(p4, h3t[:, 1, ts(bc, 128)], w4t[:, 1, :],
                                 start=False, stop=True)
                # fused acc = p4 * gate + acc on the vector engine
                nc.vector.scalar_tensor_tensor(acc[:, bc, :], p4,
                                               gates[:, bc, e:e + 1],
                                               acc[:, bc, :],
                                               ALU.mult, ALU.add)
                if e == E - 1:
                    # stream results out as each chunk finalizes to hide the
                    # store tail behind the remaining layer-4 chunks
                    eng = nc.sync if bc % 2 == 0 else nc.gpsimd
                    eng.dma_start(out=out_d.ap()[ts(bc, 128), :], in_=acc[:, bc, :])

    nc.compile()
    return nc


def _tile128(w):
    """[K, N] -> [128, K//128, N] with per-partition-contiguous bytes."""
    k, n = w.shape
    return np.ascontiguousarray(w.reshape(k // 128, 128, n).transpose(1, 0, 2))


def _fold(inputs):
    """Fold BatchNorms into next-layer weights/biases (float64 for exactness)."""
    f = {k: np.asarray(v, dtype=np.float64) for k, v in inputs.items()}

    def sb(g, b, m, v):
        s = g / np.sqrt(v + EPS)
        return s, b - m * s

    out = {}
    # gate
    sg1, tg1 = sb(f["gbn1_g"], f["gbn1_b"], f["gbn1_m"], f["gbn1_v"])
    sg2, tg2 = sb(f["gbn2_g"], f["gbn2_b"], f["gbn2_m"], f["gbn2_v"])
    gw1t = _tile128(f["gw1"])                     # [128, DCH, 256]
    gb1c = f["gb1"]
    gw2t = _tile128(sg1[:, None] * f["gw2"])      # [128, 2, 128]
    gb2c = f["gb2"] + tg1 @ f["gw2"]
    gw3t = sg2[:, None] * f["gw3"]                # [128, E]
    gb3r = f["gb3"] + tg2 @ f["gw3"]
    out["_gw1t"] = gw1t
    # experts
    s1, t1 = sb(f["ebn1_g"], f["ebn1_b"], f["ebn1_m"], f["ebn1_v"])   # [E,H]
    s2, t2 = sb(f["ebn2_g"], f["ebn2_b"], f["ebn2_m"], f["ebn2_v"])   # [E,H]
    s3, t3 = sb(f["ebn3_g"], f["ebn3_b"], f["ebn3_m"], f["ebn3_v"])   # [E,H/2]
    out["w1"] = np.stack([_tile128(f["ew1"][e]) for e in range(E)])
    b1 = f["eb1"]                                                     # [E,H]
    out["w2"] = np.stack([_tile128(s1[e][:, None] * f["ew2"][e]) for e in range(E)])
    b2 = f["eb2"] + np.einsum("eh,eho->eo", t1, f["ew2"])
    out["w3"] = np.stack([_tile128(s2[e][:, None] * f["ew3"][e]) for e in range(E)])
    b3 = f["eb3"] + np.einsum("eh,eho->eo", t2, f["ew3"])
    out["w4"] = np.stack([_tile128(s3[e][:, None] * f["ew4"][e]) for e in range(E)])
    b4 = f["eb4"] + np.einsum("eh,eho->eo", t3, f["ew4"])
    # packed activation-bias columns: [E, 128, 10]
    eb = np.zeros((E, 128, 10))
    eb[:, :, 0:4] = b1.reshape(E, 4, 128).transpose(0, 2, 1)
    eb[:, :, 4:8] = b2.reshape(E, 4, 128).transpose(0, 2, 1)
    eb[:, :, 8:10] = b3.reshape(E, 2, 128).transpose(0, 2, 1)
    out["eb"] = eb.transpose(1, 0, 2)             # [128, E, 10]
    pkr = np.zeros((128, 664))
    pkr[:1, 0:128] = 1.0                          # ones row
    pkr[:, 128:384] = gw2t.reshape(128, 256)
    pkr[:, 384:396] = gw3t
    pkr[:1, 396:408] = gb3r
    pkr[:E, 408:664] = b4
    out["pkr"] = pkr
    pkf = np.zeros((128, 131))
    pkf[:, 0:2] = gb1c.reshape(2, 128).T
    pkf[:, 2:3] = gb2c.reshape(1, 128).T
    pkf[:, 3:131] = np.eye(128)
    out["pkf"] = pkf

    res = {}
    for k, v in out.items():
        dt = np.float32 if k in ("eb", "pkf") else ml_dtypes.bfloat16
        res[k] = np.ascontiguousarray(v, dtype=dt) if k != "_gw1t" else v
    return res


_CACHE = {}


def build_in_maps(inputs):
    w = _fold(inputs)
    gw1t = w.pop("_gw1t").transpose(1, 0, 2)                            # [DCH, 128, 256]
    xt_full = np.asarray(inputs["x"], dtype=np.float32).T               # [D, B]
    in_maps = []
    for c in range(NCORES):
        m = dict(w)
        xtg = np.empty((DCH, 128, BL + 256), dtype=ml_dtypes.bfloat16)
        xtg[:, :, :BL] = xt_full[:, c * BL:(c + 1) * BL].reshape(DCH, 128, BL).astype(ml_dtypes.bfloat16)
        xtg[:, :, BL:] = gw1t.astype(ml_dtypes.bfloat16)
        m["xt"] = xtg
        in_maps.append(m)

    return in_maps


def kernel(**inputs) -> np.ndarray:
    if "nc" not in _CACHE:
        _CACHE["nc"] = _build_bass()
    nc = _CACHE["nc"]

    in_maps = build_in_maps(inputs)
    res = run_bass_kernel_spmd(nc, in_maps, core_ids=list(range(NCORES)))
    return np.concatenate([r["out"] for r in res.results], axis=0)


# revision 9
# speedup vs baseline: 1.1147x; 1.0138x over previous
"""Trainium2 Bass kernel for nn_MixtureOfExperts (B=8192, D=1024, E=12, H=512, O=256).

Strategy:
- Data-parallel over 8 NeuronCores: each core processes 1024 rows of x; all
  weights replicated. Host gathers/concats core outputs.
- Host-side prep: eval-mode BatchNorm (which follows each ReLU) is folded into
  the NEXT layer's weights and bias:  bn(relu(z)) = s*relu(z) + t  with
  s = g/sqrt(v+eps) > 0, t = b - m*s, so
      bn(relu(z)) @ W + c  ==  relu(z) @ (diag(s) W) + (c + t @ W).
  x is pre-transposed and all weights pre-tiled on host into the exact SBUF
  layout ([128 part, chunk, free] with per-partition-contiguous DRAM bytes) so
  every big DMA is a fully contiguous copy.
- All matmul operands in bf16 (same PE stream rate as fp32r, half the DMA
  bytes and LDWEIGHTS time); PSUM accumulation and bias/softmax math in fp32.
- Layers 1-3 feature-major; layer 4 batch-major (stationary = h3T slice); gate
  prob applied as per-partition scalar on ScalarE, experts accumulated on
  VectorE into acc, which is pre-initialized with sum_e gate_e * bias4_e
  computed via a PE-transposed-gates matmul against the bias matrix.
- Softmax-dependent PE work (gate transposes + acc init) is deferred until
  after expert-0 layer 1 so the PE never stalls on the softmax chain; the
  last expert's layer-4 results are DMA'd out per batch-chunk to hide the
  output-store tail behind compute.
"""

import numpy as np
import ml_dtypes
from contextlib import ExitStack

import concourse.bass as bass
import concourse.mybir as mybir
import concourse.tile as tile
from concourse import bacc
from concourse.bass import ts
from concourse.bass_utils import run_bass_kernel_spmd

B, D, E, H, O = 8192, 1024, 12, 512, 256
NCORES = 8
BL = B // NCORES          # 1024 batch rows per core
EPS = 1e-5
F32 = mybir.dt.float32
BF16 = mybir.dt.bfloat16
AF = mybir.ActivationFunctionType
ALU = mybir.AluOpType
AX = mybir.AxisListType

DCH = D // 128            # 8  d-chunks
H1CH = H // 128           # 4  h1-chunks
H3CH = (H // 2) // 128    # 2  h3-chunks
BCH = BL // 128           # 8  b-chunks of 128
BH = BL // 512            # 2  b-halves of 512
NB = 512                  # moving free dim for layers 1-3


def _build_bass():
    nc = bacc.Bacc("TRN2", target_bir_lowering=False, debug=False,
                   enable_asserts=False, num_devices=NCORES)

    # DRAM tensors. Weight tensors are host-pre-tiled to [.., 128, ch, free]
    # so the per-expert slab is contiguous and DMAs coalesce. All matmul
    # operand tensors are bf16.
    xt_d = nc.dram_tensor("xt", [DCH, 128, BL + 256], BF16, kind="ExternalInput")
    w1_d = nc.dram_tensor("w1", [E, 128, DCH, H], BF16, kind="ExternalInput")
    w2_d = nc.dram_tensor("w2", [E, 128, H1CH, H], BF16, kind="ExternalInput")
    w3_d = nc.dram_tensor("w3", [E, 128, H1CH, H // 2], BF16, kind="ExternalInput")
    w4_d = nc.dram_tensor("w4", [E, 128, H3CH, O], BF16, kind="ExternalInput")
    eb_d = nc.dram_tensor("eb", [128, E, 10], F32, kind="ExternalInput")
    # packed small constants:
    #   pkr (bf16) cols: [0:128 ones | 128:384 gw2 | 384:396 gw3 | 396:408 gb3 | 408:664 bmat]
    #   pkf (f32)  cols: [0:2 gb1 | 2:3 gb2 | 3:131 ident]
    pkr_d = nc.dram_tensor("pkr", [128, 664], BF16, kind="ExternalInput")
    pkf_d = nc.dram_tensor("pkf", [128, 131], F32, kind="ExternalInput")
    out_d = nc.dram_tensor("out", [BL, O], F32, kind="ExternalOutput")

    with tile.TileContext(nc) as tc, ExitStack() as ctx:
        const = ctx.enter_context(tc.tile_pool(name="const", bufs=1))
        gatep = ctx.enter_context(tc.tile_pool(name="gatep", bufs=1))
        gtmp = ctx.enter_context(tc.tile_pool(name="gtmp", bufs=2))
        wpool = ctx.enter_context(tc.tile_pool(name="wpool", bufs=3))
        actp = ctx.enter_context(tc.tile_pool(name="actp", bufs=1))
        accp = ctx.enter_context(tc.tile_pool(name="accp", bufs=1))
        psA = ctx.enter_context(tc.tile_pool(name="psA", bufs=4, space="PSUM"))
        ps4 = ctx.enter_context(tc.tile_pool(name="ps4", bufs=4, space="PSUM"))

        # ---- constants / full-lifetime tiles. Both hardware DGE queues
        # (sync + scalar) are used; the gpsimd software DGE is avoided —
        # it ramps slowly and pays a multi-us drain at kernel end ----
        pkf = const.tile([128, 131], F32)
        nc.scalar.dma_start(out=pkf, in_=pkf_d.ap())
        pkr = const.tile([128, 664], BF16)
        nc.sync.dma_start(out=pkr, in_=pkr_d.ap())
        xtg = const.tile([128, DCH, BL + 256], BF16)
        for dc in range(DCH):
            eng = nc.sync if dc % 2 == 0 else nc.scalar
            eng.dma_start(out=xtg[:, dc], in_=xt_d.ap()[dc])
        ebt = const.tile([128, E, 10], F32)
        nc.scalar.dma_start(out=ebt, in_=eb_d.ap())

        gw2 = pkr[:, 128:384].rearrange("p (c m) -> p c m", c=2)
        gw3 = pkr[:, 384:396]
        gb3 = pkr[:1, 396:408]
        ones = pkr[:1, 0:128]
        bmat = pkr[:E, 408:664]
        gb1 = pkf[:, 0:2]
        gb2 = pkf[:, 2:3]
        ident = pkf[:, 3:131]
        acc = accp.tile([128, BCH, O], F32)

        # ---- PE warmup on the first-arriving constant tile: starts the
        # HAM clock ramp before the gate matmuls are data-ready ----
        for r in range(6):
            wps = ps4.tile([128, 128], F32, tag="p4", name="warm")
            nc.tensor.matmul(wps, ident, ident, start=True, stop=True)

        # ---- gate network (layers 1-3; softmax tail deferred) ----
        g1t = gatep.tile([128, 2, BL], BF16)
        g2t = gatep.tile([128, BL], BF16)
        gates = gatep.tile([128, BCH, E], F32)
        for bh in range(BH):
            for hc in range(2):
                ps = psA.tile([128, NB], F32)
                for dc in range(DCH):
                    nc.tensor.matmul(ps, xtg[:, dc, BL + hc * 128:BL + hc * 128 + 128],
                                     xtg[:, dc, ts(bh, NB)],
                                     start=(dc == 0), stop=(dc == DCH - 1))
                nc.scalar.activation(g1t[:, hc, ts(bh, NB)], ps, AF.Relu,
                                     bias=gb1[:, hc:hc + 1])
            ps = psA.tile([128, NB], F32)
            for kc in range(2):
                nc.tensor.matmul(ps, gw2[:, kc, :], g1t[:, kc, ts(bh, NB)],
                                 start=(kc == 0), stop=(kc == 1))
            nc.scalar.activation(g2t[:, ts(bh, NB)], ps, AF.Relu, bias=gb2[:, 0:1])
        psgall = ps4.tile([128, BCH, E], F32, tag="p4", name="psgall")
        for bc in range(BCH):
            nc.tensor.matmul(psgall[:, bc, :], g2t[:, ts(bc, 128)], gw3,
                             start=True, stop=False)
            nc.tensor.matmul(psgall[:, bc, :], ones[:1, :], gb3[:1, :],
                             start=False, stop=True)
        exall = gatep.tile([128, BCH, E], F32)
        nc.scalar.activation(exall, psgall, AF.Exp)
        sms = gtmp.tile([128, BCH], F32)
        nc.vector.tensor_reduce(sms, exall, AX.X, ALU.add)
        rcs = gtmp.tile([128, BCH], F32)
        nc.vector.reciprocal(rcs, sms)
        for bc in range(BCH):
            nc.scalar.activation(gates[:, bc, :], exall[:, bc, :], AF.Copy,
                                 scale=rcs[:, bc:bc + 1])
        gTall = gatep.tile([E, BCH, 128], BF16)

        # ---- experts ----
        for e in range(E):
            w1t = wpool.tile([128, DCH, H], BF16)
            nc.sync.dma_start(out=w1t[:, :DCH // 2], in_=w1_d.ap()[e, :, :DCH // 2])
            nc.scalar.dma_start(out=w1t[:, DCH // 2:], in_=w1_d.ap()[e, :, DCH // 2:])
            w2t = wpool.tile([128, H1CH, H], BF16)
            nc.sync.dma_start(out=w2t, in_=w2_d.ap()[e])
            w3t = wpool.tile([128, H1CH, H // 2], BF16)
            nc.scalar.dma_start(out=w3t, in_=w3_d.ap()[e])
            w4t = wpool.tile([128, H3CH, O], BF16)
            nc.scalar.dma_start(out=w4t, in_=w4_d.ap()[e])

            h1t = actp.tile([128, H1CH, BL], BF16)
            h2t = actp.tile([128, H1CH, BL], BF16)
            h3t = actp.tile([128, H3CH, BL], BF16)

            for bh in range(BH):            # layer 1: [1024] -> [512]
                for hc in range(H1CH):
                    ps = psA.tile([128, NB], F32)
                    for dc in range(DCH):
                        nc.tensor.matmul(ps, w1t[:, dc, ts(hc, 128)],
                                         xtg[:, dc, ts(bh, NB)],
                                         start=(dc == 0), stop=(dc == DCH - 1))
                    nc.vector.tensor_scalar(h1t[:, hc, ts(bh, NB)], ps,
                                            ebt[:, e, hc:hc + 1], 0.0,
                                            ALU.add, ALU.max)

            if e == 0:
                # softmax-dependent PE tail, scheduled here so the PE queue
                # never waits on the scalar/vector softmax chain: transpose
                # gates and init acc with the gate-weighted layer-4 bias
                # (acc = gates @ B).
                for bc in range(BCH):
                    gps = ps4.tile([E, 128], F32, tag="p4", name="gps")
                    nc.tensor.transpose(gps, gates[:, bc, :], ident)
                    nc.scalar.activation(gTall[:, bc, :], gps, AF.Copy)
                for bc in range(BCH):
                    bps = ps4.tile([128, O], F32, tag="p4")
                    nc.tensor.matmul(bps, gTall[:, bc, :], bmat, start=True, stop=True)
                    nc.vector.tensor_copy(acc[:, bc, :], bps)

            for bh in range(BH):            # layer 2: [512] -> [512]
                for hc in range(H1CH):
                    ps = psA.tile([128, NB], F32)
                    for kc in range(H1CH):
                        nc.tensor.matmul(ps, w2t[:, kc, ts(hc, 128)], h1t[:, kc, ts(bh, NB)],
                                         start=(kc == 0), stop=(kc == H1CH - 1))
                    nc.scalar.activation(h2t[:, hc, ts(bh, NB)], ps, AF.Relu,
                                         bias=ebt[:, e, 4 + hc:5 + hc])
            for bh in range(BH):            # layer 3: [512] -> [256]
                for hc in range(H3CH):
                    ps = psA.tile([128, NB], F32)
                    for kc in range(H1CH):
                        nc.tensor.matmul(ps, w3t[:, kc, ts(hc, 128)], h2t[:, kc, ts(bh, NB)],
                                         start=(kc == 0), stop=(kc == H1CH - 1))
                    nc.scalar.activation(h3t[:, hc, ts(bh, NB)], ps, AF.Relu,
                                         bias=ebt[:, e, 8 + hc:9 + hc])
            for bc in range(BCH):           # layer 4 + gated accumulation
                p4 = ps4.tile([128, O], F32, tag="p4")
                nc.tensor.matmul(p4, h3t[:, 0, ts(bc, 128)], w4t[:, 0, :],
                                 start=True, stop=False)
                nc.tensor.matmul(p4, h3t[:, 1, ts(bc, 128)], w4t[:, 1, :],
                                 start=False, stop=True)
                # fused acc = p4 * gate + acc on the vector engine
                nc.vector.scalar_tensor_tensor(acc[:, bc, :], p4,
                                               gates[:, bc, e:e + 1],
                                               acc[:, bc, :],
                                               ALU.mult, ALU.add)
                if e == E - 1:
                    # stream results out as each chunk finalizes to hide the
                    # store tail behind the remaining layer-4 chunks
                    eng = nc.sync if bc % 2 == 0 else nc.scalar
                    eng.dma_start(out=out_d.ap()[ts(bc, 128), :], in_=acc[:, bc, :])

    nc.compile()
    return nc


def _tile128(w):
    """[K, N] -> [128, K//128, N] with per-partition-contiguous bytes."""
    k, n = w.shape
    return np.ascontiguousarray(w.reshape(k // 128, 128, n).transpose(1, 0, 2))


def _fold(inputs):
    """Fold BatchNorms into next-layer weights/biases (float64 for exactness)."""
    f = {k: np.asarray(v, dtype=np.float64) for k, v in inputs.items()}

    def sb(g, b, m, v):
        s = g / np.sqrt(v + EPS)
        return s, b - m * s

    out = {}
    # gate
    sg1, tg1 = sb(f["gbn1_g"], f["gbn1_b"], f["gbn1_m"], f["gbn1_v"])
    sg2, tg2 = sb(f["gbn2_g"], f["gbn2_b"], f["gbn2_m"], f["gbn2_v"])
    gw1t = _tile128(f["gw1"])                     # [128, DCH, 256]
    gb1c = f["gb1"]
    gw2t = _tile128(sg1[:, None] * f["gw2"])      # [128, 2, 128]
    gb2c = f["gb2"] + tg1 @ f["gw2"]
    gw3t = sg2[:, None] * f["gw3"]                # [128, E]
    gb3r = f["gb3"] + tg2 @ f["gw3"]
    out["_gw1t"] = gw1t
    # experts
    s1, t1 = sb(f["ebn1_g"], f["ebn1_b"], f["ebn1_m"], f["ebn1_v"])   # [E,H]
    s2, t2 = sb(f["ebn2_g"], f["ebn2_b"], f["ebn2_m"], f["ebn2_v"])   # [E,H]
    s3, t3 = sb(f["ebn3_g"], f["ebn3_b"], f["ebn3_m"], f["ebn3_v"])   # [E,H/2]
    out["w1"] = np.stack([_tile128(f["ew1"][e]) for e in range(E)])
    b1 = f["eb1"]                                                     # [E,H]
    out["w2"] = np.stack([_tile128(s1[e][:, None] * f["ew2"][e]) for e in range(E)])
    b2 = f["eb2"] + np.einsum("eh,eho->eo", t1, f["ew2"])
    out["w3"] = np.stack([_tile128(s2[e][:, None] * f["ew3"][e]) for e in range(E)])
    b3 = f["eb3"] + np.einsum("eh,eho->eo", t2, f["ew3"])
    out["w4"] = np.stack([_tile128(s3[e][:, None] * f["ew4"][e]) for e in range(E)])
    b4 = f["eb4"] + np.einsum("eh,eho->eo", t3, f["ew4"])
    # packed activation-bias columns: [E, 128, 10]
    eb = np.zeros((E, 128, 10))
    eb[:, :, 0:4] = b1.reshape(E, 4, 128).transpose(0, 2, 1)
    eb[:, :, 4:8] = b2.reshape(E, 4, 128).transpose(0, 2, 1)
    eb[:, :, 8:10] = b3.reshape(E, 2, 128).transpose(0, 2, 1)
    out["eb"] = eb.transpose(1, 0, 2)             # [128, E, 10]
    pkr = np.zeros((128, 664))
    pkr[:1, 0:128] = 1.0                          # ones row
    pkr[:, 128:384] = gw2t.reshape(128, 256)
    pkr[:, 384:396] = gw3t
    pkr[:1, 396:408] = gb3r
    pkr[:E, 408:664] = b4
    out["pkr"] = pkr
    pkf = np.zeros((128, 131))
    pkf[:, 0:2] = gb1c.reshape(2, 128).T
    pkf[:, 2:3] = gb2c.reshape(1, 128).T
    pkf[:, 3:131] = np.eye(128)
    out["pkf"] = pkf

    res = {}
    for k, v in out.items():
        dt = np.float32 if k in ("eb", "pkf") else ml_dtypes.bfloat16
        res[k] = np.ascontiguousarray(v, dtype=dt) if k != "_gw1t" else v
    return res


_CACHE = {}


def build_in_maps(inputs):
    w = _fold(inputs)
    gw1t = w.pop("_gw1t").transpose(1, 0, 2)                            # [DCH, 128, 256]
    xt_full = np.asarray(inputs["x"], dtype=np.float32).T               # [D, B]
    in_maps = []
    for c in range(NCORES):
        m = dict(w)
        xtg = np.empty((DCH, 128, BL + 256), dtype=ml_dtypes.bfloat16)
        xtg[:, :, :BL] = xt_full[:, c * BL:(c + 1) * BL].reshape(DCH, 128, BL).astype(ml_dtypes.bfloat16)
        xtg[:, :, BL:] = gw1t.astype(ml_dtypes.bfloat16)
        m["xt"] = xtg
        in_maps.append(m)

    return in_maps


def kernel(**inputs) -> np.ndarray:
    if "nc" not in _CACHE:
        _CACHE["nc"] = _build_bass()
    nc = _CACHE["nc"]

    in_maps = build_in_maps(inputs)
    res = run_bass_kernel_spmd(nc, in_maps, core_ids=list(range(NCORES)))
    return np.concatenate([r["out"] for r in res.results], axis=0)


# revision 13
# speedup vs baseline: 1.1240x; 1.0083x over previous
"""Trainium2 Bass kernel for nn_MixtureOfExperts (B=8192, D=1024, E=12, H=512, O=256).

Strategy:
- Data-parallel over 8 NeuronCores: each core processes 1024 rows of x; all
  weights replicated. Host gathers/concats core outputs.
- Host-side prep: eval-mode BatchNorm (which follows each ReLU) is folded into
  the NEXT layer's weights and bias:  bn(relu(z)) = s*relu(z) + t  with
  s = g/sqrt(v+eps) > 0, t = b - m*s, so
      bn(relu(z)) @ W + c  ==  relu(z) @ (diag(s) W) + (c + t @ W).
  x is pre-transposed and all weights pre-tiled on host into the exact SBUF
  layout ([128 part, chunk, free] with per-partition-contiguous DRAM bytes) so
  every big DMA is a fully contiguous copy.
- All matmul operands in bf16 (same PE stream rate as fp32r, half the DMA
  bytes and LDWEIGHTS time); PSUM accumulation and bias/softmax math in fp32.
- Layers 1-3 feature-major; layer 4 batch-major (stationary = h3T slice); gate
  prob applied as per-partition scalar on ScalarE, experts accumulated on
  VectorE into acc, which is pre-initialized with sum_e gate_e * bias4_e
  computed via a PE-transposed-gates matmul against the bias matrix.
- Softmax-dependent PE work (gate transposes + acc init) is deferred until
  after expert-0 layer 1 so the PE never stalls on the softmax chain; the
  last expert's layer-4 results are DMA'd out per batch-chunk to hide the
  output-store tail behind compute.
"""

import numpy as np
import ml_dtypes
from contextlib import ExitStack

import concourse.bass as bass
import concourse.mybir as mybir
import concourse.tile as tile
from concourse import bacc
from concourse.bass import ts
from concourse.bass_utils import run_bass_kernel_spmd

B, D, E, H, O = 8192, 1024, 12, 512, 256
NCORES = 8
BL = B // NCORES          # 1024 batch rows per core
EPS = 1e-5
F32 = mybir.dt.float32
BF16 = mybir.dt.bfloat16
AF = mybir.ActivationFunctionType
ALU = mybir.AluOpType
AX = mybir.AxisListType

DCH = D // 128            # 8  d-chunks
H1CH = H // 128           # 4  h1-chunks
H3CH = (H // 2) // 128    # 2  h3-chunks
BCH = BL // 128           # 8  b-chunks of 128
BH = BL // 512            # 2  b-halves of 512
NB = 512                  # moving free dim for layers 1-3


def _build_bass():
    nc = bacc.Bacc("TRN2", target_bir_lowering=False, debug=False,
                   enable_asserts=False, num_devices=NCORES)

    # DRAM tensors. Weight tensors are host-pre-tiled to [.., 128, ch, free]
    # so the per-expert slab is contiguous and DMAs coalesce. All matmul
    # operand tensors are bf16.
    xt_d = nc.dram_tensor("xt", [DCH, 128, BL + 256], BF16, kind="ExternalInput")
    w1_d = nc.dram_tensor("w1", [E, 128, DCH, H], BF16, kind="ExternalInput")
    # w2|w3|w4 packed along the free dim: [0:2048 w2 | 2048:3072 w3 | 3072:3584 w4]
    wr_d = nc.dram_tensor("wr", [E, 128, 3584], BF16, kind="ExternalInput")
    # packed small constants:
    #   pkr (bf16) cols: [0:128 ones | 128:384 gw2 | 384:396 gw3 | 396:408 gb3 | 408:664 bmat]
    #   pkf (f32)  cols: [0:2 gb1 | 2:3 gb2 | 3:131 ident | 131:251 eb]
    pkr_d = nc.dram_tensor("pkr", [128, 664], BF16, kind="ExternalInput")
    pkf_d = nc.dram_tensor("pkf", [128, 251], F32, kind="ExternalInput")
    out_d = nc.dram_tensor("out", [BL, O], F32, kind="ExternalOutput")

    with tile.TileContext(nc) as tc, ExitStack() as ctx:
        const = ctx.enter_context(tc.tile_pool(name="const", bufs=1))
        gatep = ctx.enter_context(tc.tile_pool(name="gatep", bufs=1))
        gtmp = ctx.enter_context(tc.tile_pool(name="gtmp", bufs=2))
        wpool = ctx.enter_context(tc.tile_pool(name="wpool", bufs=3))
        actp = ctx.enter_context(tc.tile_pool(name="actp", bufs=1))
        accp = ctx.enter_context(tc.tile_pool(name="accp", bufs=1))
        psA = ctx.enter_context(tc.tile_pool(name="psA", bufs=4, space="PSUM"))
        ps4 = ctx.enter_context(tc.tile_pool(name="ps4", bufs=4, space="PSUM"))

        # ---- constants / full-lifetime tiles. Both hardware DGE queues
        # (sync + scalar) are used; the gpsimd software DGE is avoided —
        # it ramps slowly and pays a multi-us drain at kernel end. x chunks
        # go first on each queue so the gate matmuls can start earliest ----
        xtg = const.tile([128, DCH, BL + 256], BF16)
        for dc in range(DCH):
            eng = nc.sync if dc % 2 == 0 else nc.scalar
            eng.dma_start(out=xtg[:, dc], in_=xt_d.ap()[dc])
        pkr = const.tile([128, 664], BF16)
        nc.sync.dma_start(out=pkr, in_=pkr_d.ap())
        pkf = const.tile([128, 251], F32)
        nc.scalar.dma_start(out=pkf, in_=pkf_d.ap())

        gw2 = pkr[:, 128:384].rearrange("p (c m) -> p c m", c=2)
        gw3 = pkr[:, 384:396]
        gb3 = pkr[:1, 396:408]
        ones = pkr[:1, 0:128]
        bmat = pkr[:E, 408:664]
        gb1 = pkf[:, 0:2]
        gb2 = pkf[:, 2:3]
        ident = pkf[:, 3:131]
        ebt = pkf[:, 131:251].rearrange("p (e c) -> p e c", e=E)
        acc = accp.tile([128, BCH, O], F32)

        # ---- PE warmup on a locally-memset tile: starts the HAM clock
        # ramp while the startup DMAs are still in flight ----
        scr = const.tile([128, 128], F32)
        nc.vector.memset(scr, 0.0)
        for r in range(6):
            wps = ps4.tile([128, 128], F32, tag="p4", name="warm")
            nc.tensor.matmul(wps, scr, scr, start=True, stop=True)

        # ---- gate network (layers 1-3; softmax tail deferred) ----
        g1t = gatep.tile([128, 2, BL], BF16)
        g2t = gatep.tile([128, BL], BF16)
        gates = gatep.tile([128, BCH, E], F32)
        for bh in range(BH):
            for hc in range(2):
                ps = psA.tile([128, NB], F32)
                for dc in range(DCH):
                    nc.tensor.matmul(ps, xtg[:, dc, BL + hc * 128:BL + hc * 128 + 128],
                                     xtg[:, dc, ts(bh, NB)],
                                     start=(dc == 0), stop=(dc == DCH - 1))
                nc.scalar.activation(g1t[:, hc, ts(bh, NB)], ps, AF.Relu,
                                     bias=gb1[:, hc:hc + 1])
            ps = psA.tile([128, NB], F32)
            for kc in range(2):
                nc.tensor.matmul(ps, gw2[:, kc, :], g1t[:, kc, ts(bh, NB)],
                                 start=(kc == 0), stop=(kc == 1))
            nc.scalar.activation(g2t[:, ts(bh, NB)], ps, AF.Relu, bias=gb2[:, 0:1])
        psgall = ps4.tile([128, BCH, E], F32, tag="p4", name="psgall")
        for bc in range(BCH):
            nc.tensor.matmul(psgall[:, bc, :], g2t[:, ts(bc, 128)], gw3,
                             start=True, stop=False)
            nc.tensor.matmul(psgall[:, bc, :], ones[:1, :], gb3[:1, :],
                             start=False, stop=True)
        exall = gatep.tile([128, BCH, E], F32)
        nc.scalar.activation(exall, psgall, AF.Exp)
        sms = gtmp.tile([128, BCH], F32)
        nc.vector.tensor_reduce(sms, exall, AX.X, ALU.add)
        rcs = gtmp.tile([128, BCH], F32)
        nc.vector.reciprocal(rcs, sms)
        for bc in range(BCH):
            nc.scalar.activation(gates[:, bc, :], exall[:, bc, :], AF.Copy,
                                 scale=rcs[:, bc:bc + 1])
        gTall = gatep.tile([E, BCH, 128], BF16)

        # ---- experts ----
        for e in range(E):
            w1t = wpool.tile([128, DCH, H], BF16)
            nc.sync.dma_start(out=w1t[:, :DCH // 2], in_=w1_d.ap()[e, :, :DCH // 2])
            nc.scalar.dma_start(out=w1t[:, DCH // 2:], in_=w1_d.ap()[e, :, DCH // 2:])
            wrt = wpool.tile([128, 3584], BF16)
            nc.sync.dma_start(out=wrt, in_=wr_d.ap()[e])
            w2t = wrt[:, 0:2048].rearrange("p (c m) -> p c m", c=H1CH)
            w3t = wrt[:, 2048:3072].rearrange("p (c m) -> p c m", c=H1CH)
            w4t = wrt[:, 3072:3584].rearrange("p (c m) -> p c m", c=H3CH)

            h1t = actp.tile([128, H1CH, BL], BF16)
            h2t = actp.tile([128, H1CH, BL], BF16)
            h3t = actp.tile([128, H3CH, BL], BF16)

            for bh in range(BH):            # layer 1: [1024] -> [512]
                for hc in range(H1CH):
                    ps = psA.tile([128, NB], F32)
                    for dc in range(DCH):
                        nc.tensor.matmul(ps, w1t[:, dc, ts(hc, 128)],
                                         xtg[:, dc, ts(bh, NB)],
                                         start=(dc == 0), stop=(dc == DCH - 1))
                    nc.vector.tensor_scalar(h1t[:, hc, ts(bh, NB)], ps,
                                            ebt[:, e, hc:hc + 1], 0.0,
                                            ALU.add, ALU.max)

            if e == 0:
                # softmax-dependent PE tail, scheduled here so the PE queue
                # never waits on the scalar/vector softmax chain: transpose
                # gates and init acc with the gate-weighted layer-4 bias
                # (acc = gates @ B).
                for bc in range(BCH):
                    gps = ps4.tile([E, 128], F32, tag="p4", name="gps")
                    nc.tensor.transpose(gps, gates[:, bc, :], ident)
                    nc.scalar.activation(gTall[:, bc, :], gps, AF.Copy)
                for bc in range(BCH):
                    bps = ps4.tile([128, O], F32, tag="p4")
                    nc.tensor.matmul(bps, gTall[:, bc, :], bmat, start=True, stop=True)
                    nc.vector.tensor_copy(acc[:, bc, :], bps)

            for bh in range(BH):            # layer 2: [512] -> [512]
                for hc in range(H1CH):
                    ps = psA.tile([128, NB], F32)
                    for kc in range(H1CH):
                        nc.tensor.matmul(ps, w2t[:, kc, ts(hc, 128)], h1t[:, kc, ts(bh, NB)],
                                         start=(kc == 0), stop=(kc == H1CH - 1))
                    nc.scalar.activation(h2t[:, hc, ts(bh, NB)], ps, AF.Relu,
                                         bias=ebt[:, e, 4 + hc:5 + hc])
            for bh in range(BH):            # layer 3: [512] -> [256]
                for hc in range(H3CH):
                    ps = psA.tile([128, NB], F32)
                    for kc in range(H1CH):
                        nc.tensor.matmul(ps, w3t[:, kc, ts(hc, 128)], h2t[:, kc, ts(bh, NB)],
                                         start=(kc == 0), stop=(kc == H1CH - 1))
                    nc.scalar.activation(h3t[:, hc, ts(bh, NB)], ps, AF.Relu,
                                         bias=ebt[:, e, 8 + hc:9 + hc])
            for bc in range(BCH):           # layer 4 + gated accumulation
                p4 = ps4.tile([128, O], F32, tag="p4")
                nc.tensor.matmul(p4, h3t[:, 0, ts(bc, 128)], w4t[:, 0, :],
                                 start=True, stop=False)
                nc.tensor.matmul(p4, h3t[:, 1, ts(bc, 128)], w4t[:, 1, :],
                                 start=False, stop=True)
                # fused acc = p4 * gate + acc on the vector engine
                nc.vector.scalar_tensor_tensor(acc[:, bc, :], p4,
                                               gates[:, bc, e:e + 1],
                                               acc[:, bc, :],
                                               ALU.mult, ALU.add)
                if e == E - 1:
                    # stream results out as each chunk finalizes to hide the
                    # store tail behind the remaining layer-4 chunks
                    eng = nc.sync if bc % 2 == 0 else nc.scalar
                    eng.dma_start(out=out_d.ap()[ts(bc, 128), :], in_=acc[:, bc, :])

    nc.compile()
    return nc


def _tile128(w):
    """[K, N] -> [128, K//128, N] with per-partition-contiguous bytes."""
    k, n = w.shape
    return np.ascontiguousarray(w.reshape(k // 128, 128, n).transpose(1, 0, 2))


def _fold(inputs):
    """Fold BatchNorms into next-layer weights/biases (float64 for exactness)."""
    f = {k: np.asarray(v, dtype=np.float64) for k, v in inputs.items()}

    def sb(g, b, m, v):
        s = g / np.sqrt(v + EPS)
        return s, b - m * s

    out = {}
    # gate
    sg1, tg1 = sb(f["gbn1_g"], f["gbn1_b"], f["gbn1_m"], f["gbn1_v"])
    sg2, tg2 = sb(f["gbn2_g"], f["gbn2_b"], f["gbn2_m"], f["gbn2_v"])
    gw1t = _tile128(f["gw1"])                     # [128, DCH, 256]
    gb1c = f["gb1"]
    gw2t = _tile128(sg1[:, None] * f["gw2"])      # [128, 2, 128]
    gb2c = f["gb2"] + tg1 @ f["gw2"]
    gw3t = sg2[:, None] * f["gw3"]                # [128, E]
    gb3r = f["gb3"] + tg2 @ f["gw3"]
    out["_gw1t"] = gw1t
    # experts
    s1, t1 = sb(f["ebn1_g"], f["ebn1_b"], f["ebn1_m"], f["ebn1_v"])   # [E,H]
    s2, t2 = sb(f["ebn2_g"], f["ebn2_b"], f["ebn2_m"], f["ebn2_v"])   # [E,H]
    s3, t3 = sb(f["ebn3_g"], f["ebn3_b"], f["ebn3_m"], f["ebn3_v"])   # [E,H/2]
    out["w1"] = np.stack([_tile128(f["ew1"][e]) for e in range(E)])
    b1 = f["eb1"]                                                     # [E,H]
    w2 = np.stack([_tile128(s1[e][:, None] * f["ew2"][e]) for e in range(E)])
    b2 = f["eb2"] + np.einsum("eh,eho->eo", t1, f["ew2"])
    w3 = np.stack([_tile128(s2[e][:, None] * f["ew3"][e]) for e in range(E)])
    b3 = f["eb3"] + np.einsum("eh,eho->eo", t2, f["ew3"])
    w4 = np.stack([_tile128(s3[e][:, None] * f["ew4"][e]) for e in range(E)])
    b4 = f["eb4"] + np.einsum("eh,eho->eo", t3, f["ew4"])
    out["wr"] = np.concatenate([w2.reshape(E, 128, -1), w3.reshape(E, 128, -1),
                                w4.reshape(E, 128, -1)], axis=2)      # [E, 128, 3584]
    # packed activation-bias columns: [E, 128, 10]
    eb = np.zeros((E, 128, 10))
    eb[:, :, 0:4] = b1.reshape(E, 4, 128).transpose(0, 2, 1)
    eb[:, :, 4:8] = b2.reshape(E, 4, 128).transpose(0, 2, 1)
    eb[:, :, 8:10] = b3.reshape(E, 2, 128).transpose(0, 2, 1)
    pkr = np.zeros((128, 664))
    pkr[:1, 0:128] = 1.0                          # ones row
    pkr[:, 128:384] = gw2t.reshape(128, 256)
    pkr[:, 384:396] = gw3t
    pkr[:1, 396:408] = gb3r
    pkr[:E, 408:664] = b4
    out["pkr"] = pkr
    pkf = np.zeros((128, 251))
    pkf[:, 0:2] = gb1c.reshape(2, 128).T
    pkf[:, 2:3] = gb2c.reshape(1, 128).T
    pkf[:, 3:131] = np.eye(128)
    pkf[:, 131:251] = eb.transpose(1, 0, 2).reshape(128, 120)
    out["pkf"] = pkf

    res = {}
    for k, v in out.items():
        dt = np.float32 if k == "pkf" else ml_dtypes.bfloat16
        res[k] = np.ascontiguousarray(v, dtype=dt) if k != "_gw1t" else v
    return res


_CACHE = {}


def build_in_maps(inputs):
    w = _fold(inputs)
    gw1t = w.pop("_gw1t").transpose(1, 0, 2)                            # [DCH, 128, 256]
    xt_full = np.asarray(inputs["x"], dtype=np.float32).T               # [D, B]
    in_maps = []
    for c in range(NCORES):
        m = dict(w)
        xtg = np.empty((DCH, 128, BL + 256), dtype=ml_dtypes.bfloat16)
        xtg[:, :, :BL] = xt_full[:, c * BL:(c + 1) * BL].reshape(DCH, 128, BL).astype(ml_dtypes.bfloat16)
        xtg[:, :, BL:] = gw1t.astype(ml_dtypes.bfloat16)
        m["xt"] = xtg
        in_maps.append(m)

    return in_maps


def kernel(**inputs) -> np.ndarray:
    if "nc" not in _CACHE:
        _CACHE["nc"] = _build_bass()
    nc = _CACHE["nc"]

    in_maps = build_in_maps(inputs)
    res = run_bass_kernel_spmd(nc, in_maps, core_ids=list(range(NCORES)))
    return np.concatenate([r["out"] for r in res.results], axis=0)
